# revision 1
# baseline (speedup 1.0000x reference)
"""LCNN (lattice GNN) Trainium2 kernel — 8-core SPMD.

Strategy:
  - Shard the N=100000 nodes across 8 cores (12500 each, padded to 12544 =
    98 tiles of 128 nodes).
  - Per 128-node tile, gather the 114 (=P*K) neighbor feature rows per node
    with per-column indirect DMAs (offsets [128,1], one row per partition).
    NOTE: multi-offset indirect_dma_start ([128,m] offsets) matches the
    CoreSim interpreter but is mis-lowered on HW (only the first offset per
    partition is honored; strided runs read junk) — verified 2026-08-05.
    Future speedup (~17x): segmented int16 dma_gather (InstDMAGatherAnt,
    256B-padded rows, 4 table segments to fit int16 indices).
    Cost-model (MultiCoreSim) predicted device time: 11.43 ms, dominated by
    22344 indirect-DMA issues x ~1us SWDGE fixed overhead on the Pool
    engine; compute (PE/DVE/ACT) and DMA payload are fully overlapped.
  - Block 1: gather raw x rows (3 floats) -> PE transpose -> 6 matmuls with
    W1 accumulated in PSUM -> folded BN affine -> h1 tile.
  - AllGather h1 shards into a full [100000,19] DRAM table.
  - Block 2: gather h1 rows (19 floats) -> 18 transposes/matmuls vs W2 ->
    folded BN affine -> Wc matmul -> LayerNorm -> softplus -> masked
    accumulate per-feature sums.
  - Each core outputs sum_n softplus(LN(...)) [25]; the tiny remaining head
    ([25]x[25,25] matmul, mean, [25]x[25,1]) runs on host in fp32.
"""

import sys

sys.path.insert(0, "/opt/trn_rl_repo")

import numpy as np

from concourse import bacc, mybir
import concourse.bass as bass
import concourse.tile as tile
from concourse import bass_utils
from concourse.bass import IndirectOffsetOnAxis
from concourse.masks import make_identity

# Problem constants (hardcoded per contract)
N, P, K = 100000, 6, 19
F0, F, SF = 3, 19, 25
NC = 8
SHARD = N // NC          # 12500
NT = 98                  # tiles of 128 rows
SHARD_PAD = NT * 128     # 12544
PK = P * K               # 114
BN_EPS = 1e-5
LN_EPS = 1e-5
LOG2 = 0.6931

F32 = mybir.dt.float32
I32 = mybir.dt.int32

# cst packed-constant columns
C_A1, C_D1, C_A2, C_D2 = 0, 19, 38, 57
C_BC, C_LNG, C_LNB = 76, 101, 126
C_MASK, C_ONES = 151, 152
C_ZERO, C_EPS = 153, 154
C_TOT = 155


def build_nc():
    nc = bacc.Bacc("TRN2", target_bir_lowering=False, debug=False,
                   num_devices=NC)

    x_t = nc.dram_tensor("x", [N, F0], F32, kind="ExternalInput")
    idx_t = nc.dram_tensor("idx", [SHARD_PAD, PK], I32, kind="ExternalInput")
    w1_t = nc.dram_tensor("w1", [F0 * K, F], F32, kind="ExternalInput")
    w2_t = nc.dram_tensor("w2", [F * K, F], F32, kind="ExternalInput")
    wc_t = nc.dram_tensor("wc", [F, SF], F32, kind="ExternalInput")
    cst_t = nc.dram_tensor("cst", [128, C_TOT], F32, kind="ExternalInput")
    out_t = nc.dram_tensor("out", [SF, 1], F32, kind="ExternalOutput")


    with tile.TileContext(nc) as tc:
        with (
            tc.tile_pool(name="const", bufs=1) as cpool,
            tc.tile_pool(name="gath", bufs=3) as gpool,
            tc.tile_pool(name="lhs", bufs=3) as lpool,
            tc.tile_pool(name="work", bufs=3) as wpool,
            tc.tile_pool(name="d1", bufs=1, space="DRAM") as dp1,
            tc.tile_pool(name="d2", bufs=1, space="DRAM") as dp2,
            tc.tile_pool(name="pst", bufs=3, space="PSUM") as pst,
            tc.tile_pool(name="psa", bufs=2, space="PSUM") as psa,
        ):
            # ---- constants ----
            cst = cpool.tile([128, C_TOT], F32)
            nc.sync.dma_start(cst[:], cst_t[:, :])
            w1s = cpool.tile([F0 * K, F], F32)
            nc.sync.dma_start(w1s[:], w1_t[:, :])
            w2s = cpool.tile([128, 3, F], F32)
            nc.sync.dma_start(w2s[:, 0, :], w2_t[0:128, :])
            nc.sync.dma_start(w2s[:, 1, :], w2_t[128:256, :])
            nc.sync.dma_start(w2s[:105, 2, :], w2_t[256:361, :])
            wcs = cpool.tile([F, SF], F32)
            nc.sync.dma_start(wcs[:], wc_t[:, :])
            ident = cpool.tile([128, 128], F32)
            make_identity(nc, ident[:])
            idx_sb = cpool.tile([128, NT, PK], I32)
            nc.sync.dma_start(idx_sb[:], idx_t.rearrange("(t p) k -> p t k", p=128))

            h1_shard = dp1.tile([SHARD_PAD, F], F32)
            h1_full = dp2.tile([N, F], F32)
            acc = cpool.tile([128, SF], F32)
            nc.vector.memset(acc[:], 0.0)

            # ---- block 1: h1 = A1*(sum_p X_p@W1) + D1 ----
            for t in range(NT):
                g1 = gpool.tile([128, PK, F0], F32, tag="g1")
                for j in range(PK):
                    nc.gpsimd.indirect_dma_start(
                        out=g1[:, j, :], out_offset=None,
                        in_=x_t[:, :],
                        in_offset=IndirectOffsetOnAxis(
                            ap=idx_sb[:, t, j:j + 1], axis=0),
                    )
                lh = lpool.tile([F0 * K, P * 128], F32, tag="lh1")
                for p in range(P):
                    tp = pst.tile([F0 * K, 128], F32, tag="tp")
                    nc.tensor.transpose(
                        out=tp[:],
                        in_=g1[:].rearrange("a b c -> a (b c)")[:, p * 57:(p + 1) * 57],
                        identity=ident[:])
                    nc.vector.tensor_copy(lh[:, p * 128:(p + 1) * 128], tp[:])
                ps_h = psa.tile([128, F], F32, tag="psh")
                for p in range(P):
                    nc.tensor.matmul(
                        out=ps_h[:], lhsT=lh[:, p * 128:(p + 1) * 128],
                        rhs=w1s[:], start=(p == 0), stop=(p == P - 1))
                s1 = wpool.tile([128, F], F32, tag="s1")
                nc.vector.tensor_tensor(
                    out=s1[:], in0=ps_h[:], in1=cst[:, C_A1:C_A1 + F],
                    op=mybir.AluOpType.mult)
                nc.vector.tensor_tensor(
                    out=s1[:], in0=s1[:], in1=cst[:, C_D1:C_D1 + F],
                    op=mybir.AluOpType.add)
                nc.sync.dma_start(h1_shard[t * 128:(t + 1) * 128, :], s1[:])

            # ---- AllGather h1 shards -> full table ----
            nc.gpsimd.collective_compute(
                "AllGather", mybir.AluOpType.bypass,
                replica_groups=[list(range(NC))],
                ins=[h1_shard[0:SHARD, :].opt()],
                outs=[h1_full[:, :].opt()],
            )

            # ---- block 2 + head ----
            for t in range(NT):
                g2 = gpool.tile([128, PK, F], F32, tag="g2")
                for j in range(PK):
                    nc.gpsimd.indirect_dma_start(
                        out=g2[:, j, :], out_offset=None,
                        in_=h1_full[:, :],
                        in_offset=IndirectOffsetOnAxis(
                            ap=idx_sb[:, t, j:j + 1], axis=0),
                    )
                lh2 = lpool.tile([128, 18 * 128], F32, tag="lh2")
                for p in range(P):
                    for c in range(3):
                        rows = 128 if c < 2 else 105
                        col0 = p * 361 + c * 128
                        tp2 = pst.tile([128, 128], F32, tag="tp")
                        nc.tensor.transpose(
                            out=tp2[:rows, :],
                            in_=g2[:].rearrange("a b c -> a (b c)")[:, col0:col0 + rows],
                            identity=ident[:])
                        nc.vector.tensor_copy(
                            lh2[:rows, (p * 3 + c) * 128:(p * 3 + c) * 128 + 128],
                            tp2[:rows, :])
                ps_h2 = psa.tile([128, F], F32, tag="psh")
                for p in range(P):
                    for c in range(3):
                        rows = 128 if c < 2 else 105
                        j = p * 3 + c
                        nc.tensor.matmul(
                            out=ps_h2[:],
                            lhsT=lh2[:rows, j * 128:j * 128 + 128],
                            rhs=w2s[:rows, c, :],
                            start=(j == 0), stop=(j == 17))
                s2 = wpool.tile([128, F], F32, tag="s2")
                nc.vector.tensor_tensor(
                    out=s2[:], in0=ps_h2[:], in1=cst[:, C_A2:C_A2 + F],
                    op=mybir.AluOpType.mult)
                nc.vector.tensor_tensor(
                    out=s2[:], in0=s2[:], in1=cst[:, C_D2:C_D2 + F],
                    op=mybir.AluOpType.add)
                # h2 @ Wc
                tp3 = pst.tile([F, 128], F32, tag="tp")
                nc.tensor.transpose(out=tp3[:], in_=s2[:], identity=ident[:])
                h2T = wpool.tile([F, 128], F32, tag="h2T")
                nc.vector.tensor_copy(h2T[:], tp3[:])
                ps3 = psa.tile([128, SF], F32, tag="ps3")
                nc.tensor.matmul(out=ps3[:], lhsT=h2T[:], rhs=wcs[:],
                                 start=True, stop=True)
                h3 = wpool.tile([128, SF], F32, tag="h3")
                nc.vector.tensor_tensor(
                    out=h3[:], in0=ps3[:], in1=cst[:, C_BC:C_BC + SF],
                    op=mybir.AluOpType.add)
                # LayerNorm over SF
                mu = wpool.tile([128, 1], F32, tag="mu")
                nc.vector.tensor_reduce(
                    out=mu[:], in_=h3[:], axis=mybir.AxisListType.X,
                    op=mybir.AluOpType.add)
                nc.scalar.mul(mu[:], mu[:], 1.0 / SF)
                xc = wpool.tile([128, SF], F32, tag="xc")
                nc.vector.tensor_scalar_sub(xc[:], h3[:], mu[:])
                sq = wpool.tile([128, SF], F32, tag="sq")
                var = wpool.tile([128, 1], F32, tag="var")
                nc.scalar.activation(
                    out=sq[:], in_=xc[:],
                    func=mybir.ActivationFunctionType.Square,
                    bias=cst[:, C_ZERO:C_ZERO + 1],
                    accum_out=var[:])
                lnv = wpool.tile([128, 1], F32, tag="lnv")
                nc.scalar.activation(
                    out=lnv[:], in_=var[:],
                    func=mybir.ActivationFunctionType.Ln,
                    bias=cst[:, C_EPS:C_EPS + 1], scale=1.0 / SF)
                rstd = wpool.tile([128, 1], F32, tag="rstd")
                nc.scalar.activation(
                    out=rstd[:], in_=lnv[:],
                    func=mybir.ActivationFunctionType.Exp,
                    bias=cst[:, C_ZERO:C_ZERO + 1], scale=-0.5)
                y = wpool.tile([128, SF], F32, tag="y")
                nc.vector.tensor_scalar_mul(y[:], xc[:], rstd[:])
                nc.vector.tensor_tensor(
                    out=y[:], in0=y[:], in1=cst[:, C_LNG:C_LNG + SF],
                    op=mybir.AluOpType.mult)
                nc.vector.tensor_tensor(
                    out=y[:], in0=y[:], in1=cst[:, C_LNB:C_LNB + SF],
                    op=mybir.AluOpType.add)
                ey = wpool.tile([128, SF], F32, tag="ey")
                nc.scalar.activation(
                    out=ey[:], in_=y[:],
                    func=mybir.ActivationFunctionType.Exp,
                    bias=cst[:, C_ZERO:C_ZERO + 1])
                sp = wpool.tile([128, SF], F32, tag="sp")
                nc.scalar.activation(
                    out=sp[:], in_=ey[:],
                    func=mybir.ActivationFunctionType.Ln,
                    bias=cst[:, C_ONES:C_ONES + 1])
                if t == NT - 1:
                    nc.vector.tensor_scalar_mul(
                        sp[:], sp[:], cst[:, C_MASK:C_MASK + 1])
                nc.vector.tensor_tensor(
                    out=acc[:], in0=acc[:], in1=sp[:],
                    op=mybir.AluOpType.add)

            # ---- per-core feature sums: [25,1] = acc.T @ ones ----
            ps4 = psa.tile([SF, 1], F32, tag="ps3")
            nc.tensor.matmul(out=ps4[:], lhsT=acc[:],
                             rhs=cst[:, C_ONES:C_ONES + 1],
                             start=True, stop=True)
            res = wpool.tile([SF, 1], F32, tag="res")
            nc.scalar.copy(res[:], ps4[:])
            nc.sync.dma_start(out_t[:, :], res[:])

    nc.compile()
    return nc


_NC_CACHE = None


def _get_nc():
    global _NC_CACHE
    if _NC_CACHE is None:
        _NC_CACHE = build_nc()
    return _NC_CACHE


def _make_in_maps(inputs):
    x = np.ascontiguousarray(inputs["x"], dtype=np.float32)
    nbr = np.ascontiguousarray(inputs["nbr_idx"], dtype=np.int32)

    def fold(g, be, rm, rv, b):
        a = g / np.sqrt(rv + BN_EPS)
        d = P * (a * (b - rm) + be)
        return a.astype(np.float32), d.astype(np.float32)

    a1, d1 = fold(inputs["g1"], inputs["be1"], inputs["rm1"], inputs["rv1"],
                  inputs["b1"])
    a2, d2 = fold(inputs["g2"], inputs["be2"], inputs["rm2"], inputs["rv2"],
                  inputs["b2"])

    cst = np.zeros((128, C_TOT), np.float32)
    cst[:, C_A1:C_A1 + F] = a1
    cst[:, C_D1:C_D1 + F] = d1
    cst[:, C_A2:C_A2 + F] = a2
    cst[:, C_D2:C_D2 + F] = d2
    cst[:, C_BC:C_BC + SF] = inputs["bc"]
    cst[:, C_LNG:C_LNG + SF] = inputs["lng"]
    cst[:, C_LNB:C_LNB + SF] = inputs["lnb"]
    # last tile holds rows 97*128 .. 97*128+127; rows >= 12500-97*128=84 are pad
    cst[:84, C_MASK] = 1.0
    cst[:, C_ONES] = 1.0
    cst[:, C_EPS] = LN_EPS

    w1 = np.ascontiguousarray(inputs["W1"], np.float32)
    w2 = np.ascontiguousarray(inputs["W2"], np.float32)
    wc = np.ascontiguousarray(inputs["Wc"], np.float32)

    in_maps = []
    for c in range(NC):
        sl = nbr[c * SHARD:(c + 1) * SHARD].reshape(SHARD, PK)
        idx = np.zeros((SHARD_PAD, PK), np.int32)
        idx[:SHARD] = sl
        in_maps.append({
            "x": x, "idx": idx, "w1": w1, "w2": w2, "wc": wc, "cst": cst,
        })
    return in_maps


def kernel(trace=False, **inputs):
    import time as _time
    nc = _get_nc()
    in_maps = _make_in_maps(inputs)
    res = bass_utils.run_bass_kernel_spmd(
        nc, in_maps, core_ids=list(range(NC)), trace=False)
    if trace:
        t0 = _time.perf_counter()
        res = bass_utils.run_bass_kernel_spmd(
            nc, in_maps, core_ids=list(range(NC)), trace=False)
        kernel.last_wall_ns = (_time.perf_counter() - t0) * 1e9
    sums = np.stack([r["out"].reshape(SF) for r in res.results])  # [NC, SF]
    total = sums.sum(axis=0, dtype=np.float64).astype(np.float32)
    # finish head on host: h3 sums -> mean -> Wl -> Wf
    h3_sum = total - np.float32(N * LOG2)
    g = (h3_sum / np.float32(N)) @ inputs["Wl"] + inputs["bl"]
    out = g @ inputs["Wf"] + inputs["bf"]
    if trace:
        kernel.last_exec_time_ns = res.exec_time_ns
        kernel.last_results = res
    return out.astype(np.float32)

